# revision 37
# baseline (speedup 1.0000x reference)
"""Multi-head attention, tensor-parallel across 8 Trainium2 NeuronCores.

Sharding: core = (batch b, head-group g), g covering 4 heads (256 dh).
Within a core heads are processed as PAIRS using 64x128 PE row-tiling:
head A of a pair lives on SBUF partitions 0-63, head B on 64-127, so the
two K=64 scores matmuls run concurrently on array tiles T0/T8, and each
AV matmul's K=128 contraction is split into top/bot halves on T0/T8
accumulating into one PSUM bank via has_written.

exp is split between ScalarE (exact, even j-tiles) and VectorE (odd
j-tiles) using a round-to-nearest int16 Schraudolph: bf16bits(exp(x)) ~
round(x*A + B), verified exact-convert on HW; its mean ratio error is
calibrated out (softmax cancels any residual common-mode bias).

Z comes from a ones column appended to V (AV psum row 64). Normalize:
reciprocal_approx_fast on Z, DRAM-roundtrip broadcast, DVE mults; head
B's normalized block is staged and DMA'd to partitions 64-127 so the
output projection keeps K=128.

Host: shards inputs, sums the 4 head-group partials per batch, adds bo.
"""

import os
import numpy as np

DBG_AV_SINGLE = os.environ.get("DBG_AV_SINGLE", "0") == "1"
DBG_RECIP_PLAIN = os.environ.get("DBG_RECIP_PLAIN", "0") == "1"
DBG_EXP_ACT = os.environ.get("DBG_EXP_ACT", "0") == "1"

B, S, D, H = 2, 2048, 1024, 16
DK = D // H              # 64 head dim
N_CORES = 8
GROUPS = N_CORES // B    # 4 head-groups
DH = D // GROUPS         # 256 head-dims per core (4 heads)
H_CORE = DH // DK        # 4 heads per core
SCALE = 1.0 / float(np.sqrt(DK))

P = 128                  # SBUF/PSUM partitions
SC = 512                 # matmul moving-dim chunk
IB = 512                 # flash i-block
LOG2E = float(np.log2(np.e))
SCH_A = float(128.0 * SCALE * LOG2E)       # schraudolph slope
SCH_B = float(127.0 * 128.0 - 7.35)        # schraudolph bias (mean-one)


def build_nc(S=S, D=D, DH=DH, DK=DK, scale=SCALE, ib=IB):
    import concourse.bacc as bacc
    import concourse.mybir as mybir
    import concourse.tile as tile

    f32 = mybir.dt.float32
    bf16 = mybir.dt.bfloat16
    i16 = mybir.dt.int16
    Exp = mybir.ActivationFunctionType.Exp
    Ident = mybir.ActivationFunctionType.Identity
    Mult = mybir.AluOpType.mult
    Add = mybir.AluOpType.add
    cdt = bf16

    KT = D // P                    # contraction tiles for projections (8)
    NSC = S // SC                  # s chunks (4)
    HC = DH // P                   # head pairs (2)
    HPC = P // DK                  # heads per pair (2)
    JT = S // P                    # j tiles (16)
    NIB = S // ib                  # i blocks (4)
    NOUT = D // P                  # output row chunks (8)
    LAG = 2                        # AV trails scores by LAG j-steps

    nc = bacc.Bacc("TRN2", target_bir_lowering=False, debug=False)

    qT = nc.dram_tensor("qT", [D, S], cdt, kind="ExternalInput")
    kTd = nc.dram_tensor("kTd", [D, S], cdt, kind="ExternalInput")
    vT = nc.dram_tensor("vT", [D, S], cdt, kind="ExternalInput")
    wq = nc.dram_tensor("wq", [D, DH], cdt, kind="ExternalInput")
    wk = nc.dram_tensor("wk", [D, DH], cdt, kind="ExternalInput")
    wv = nc.dram_tensor("wv", [D, DH], cdt, kind="ExternalInput")
    wo = nc.dram_tensor("wo", [DH, D], cdt, kind="ExternalInput")
    bq = nc.dram_tensor("bq", [P, HC], f32, kind="ExternalInput")
    bk = nc.dram_tensor("bk", [P, HC], f32, kind="ExternalInput")
    bvb = nc.dram_tensor("bvb", [P, H_CORE, DK], f32, kind="ExternalInput")
    outT = nc.dram_tensor("outT", [D, S], cdt, kind="ExternalOutput")

    with tile.TileContext(nc) as tc:
        with (
            tc.tile_pool(name="const", bufs=1) as cpool,
            tc.tile_pool(name="pers", bufs=1) as pers,
            tc.tile_pool(name="stream", bufs=1) as stream,
            tc.tile_pool(name="psum", bufs=1, space="PSUM") as psum,
            tc.tile_pool(name="dscratch", bufs=1, space="DRAM") as dscratch,
        ):
            # ---- constants ----
            wq_sb = cpool.tile([P, KT, DH], cdt, name="wq_sb")
            wk_sb = cpool.tile([P, KT, DH], cdt, name="wk_sb")
            wv_sb = cpool.tile([P, KT, DH], cdt, name="wv_sb")
            wo_sb = cpool.tile([P, HC, D], cdt, name="wo_sb")
            bq_sb = cpool.tile([P, HC], f32, name="bq_sb")
            bk_sb = cpool.tile([P, HC], f32, name="bk_sb")
            bvb_sb = cpool.tile([P, H_CORE, DK], f32, name="bvb_sb")
            # weight/bias loads are interleaved with input-tensor loads
            # below so Q-proj can start as early as possible

            # ---- persistent activations (head-pair layout) ----
            # qt/kt pair c: rows 0-63 = head 2c (dk dims), rows 64-127 =
            # head 2c+1. v pair c: rows = j within tile, + ones column.
            qt = [pers.tile([P, S], cdt, name=f"qt{c}") for c in range(HC)]
            kt = [pers.tile([P, S], cdt, name=f"kt{c}") for c in range(HC)]
            v_c = [pers.tile([P, JT, HPC, DK + 1], cdt, name=f"v{c}")
                   for c in range(HC)]
            on_c = [pers.tile([P, S], cdt, name=f"on{c}") for c in range(HC)]

            for c in range(HC):
                nc.vector.memset(v_c[c][:, :, :, DK:DK + 1], 1.0)

            # ---- PE warmup: keep HAM busy during the initial input DMA
            # wait so the first real matmuls run at full clock ----
            wmp = psum.tile([P, 2 * SC], f32, tag="av", bufs=1, name="warm")
            wsrc = cpool.tile([P, DK], cdt, name="wsrc")
            nc.vector.memset(wsrc[:], 1.0)
            for w in range(120):
                nc.tensor.matmul(wmp[0:HPC, 0:DK], lhsT=wsrc[:, 0:HPC],
                                 rhs=wsrc[:, 0:DK], start=True, stop=True)

            # ---- projections (inputs loaded as half-row 256KB DMAs) ----
            def load_tensor(src):
                bt = stream.tile([P, KT, S], cdt, tag="big_in", bufs=2,
                                 name=f"bi_{src.name}")
                for half in range(2):
                    hs = slice(half * (S // 2), (half + 1) * (S // 2))
                    for kti in range(KT):
                        nc.sync.dma_start(bt[:, kti, hs],
                                          src[kti * P:(kti + 1) * P, hs])
                return bt

            # DMA queue is FIFO: issue loads in consumption-priority order
            nc.sync.dma_start(wq_sb[:],
                              wq[:, :].rearrange("(ko p) n -> p ko n", p=P))
            nc.sync.dma_start(bq_sb[:], bq[:, :])
            qin = load_tensor(qT)
            nc.sync.dma_start(wk_sb[:],
                              wk[:, :].rearrange("(ko p) n -> p ko n", p=P))
            nc.sync.dma_start(bk_sb[:], bk[:, :])
            kin = load_tensor(kTd)
            nc.sync.dma_start(wv_sb[:],
                              wv[:, :].rearrange("(ko p) n -> p ko n", p=P))
            nc.sync.dma_start(bvb_sb[:], bvb[:, :, :])
            vin = load_tensor(vT)
            nc.sync.dma_start(wo_sb[:],
                              wo[:, :].rearrange("(c p) n -> p c n", p=P))

            def qk_proj(bt, w_sb, b_sb, dst):
                for si in range(NSC):
                    ps = psum.tile([P, 2 * SC], f32, tag="sc", bufs=3,
                                   name=f"ps_{dst[0].name}_{si}")
                    ssl = slice(si * SC, (si + 1) * SC)
                    for c in range(HC):
                        for kti in range(KT):
                            nc.tensor.matmul(
                                ps[:, c * SC:(c + 1) * SC],
                                lhsT=w_sb[:, kti, c * P:(c + 1) * P],
                                rhs=bt[:, kti, ssl],
                                start=(kti == 0), stop=(kti == KT - 1))
                    # evac + bias: head-pair chunk c goes straight to dst[c]
                    nc.vector.tensor_add(
                        dst[0][:, ssl], ps[:, 0:SC],
                        b_sb[:, 0:1].to_broadcast((P, SC)))
                    nc.scalar.activation(
                        dst[1][:, ssl], ps[:, SC:2 * SC], Ident,
                        bias=b_sb[:, 1:2], scale=1.0)

            qk_proj(qin, wq_sb, bq_sb, qt)
            qk_proj(kin, wk_sb, bk_sb, kt)

            # ---- V projection (natural [j, dh]) ----
            for si in range(NSC):
                for sub in range(SC // P):
                    jt_idx = si * (SC // P) + sub
                    ps = psum.tile([P, 2 * SC], f32, tag="sc", bufs=3,
                                   name=f"ps_v_{jt_idx}")
                    jsl = slice(si * SC + sub * P, si * SC + (sub + 1) * P)
                    for kti in range(KT):
                        nc.tensor.matmul(
                            ps[:, 0:DH],
                            lhsT=vin[:, kti, jsl],
                            rhs=wv_sb[:, kti, :],
                            start=(kti == 0), stop=(kti == KT - 1))
                    for c in range(HC):
                        src_ap = ps[:, c * P:(c + 1) * P].rearrange(
                            "p (h d) -> p h d", d=DK)
                        dst_ap = v_c[c][:, jt_idx, :, 0:DK]
                        bias_ap = bvb_sb[:, c * HPC:(c + 1) * HPC, :]
                        nc.vector.tensor_add(dst_ap, src_ap, bias_ap)

            # ---- attention (flash over j; head pairs on T0/T8) ----
            for c in range(HC):
                for ibx in range(NIB):
                    i0 = ibx * ib
                    isl = slice(i0, i0 + ib)
                    av = psum.tile([P, 2 * SC], f32, tag="av", bufs=1,
                                   name=f"av_{c}_{ibx}")
                    e_ts = {}
                    # batch 2 j-steps per group: 4 scores MMs (64x128 mode)
                    # then 4 AV MMs (128x128) -> fewer mode-switch drains
                    for jg in range(JT // 2 + 1):
                        for sub in range(2):
                            jt = 2 * jg + sub
                            if jt >= JT:
                                continue
                            sct = psum.tile([P, 2 * SC], f32, tag="sc",
                                            bufs=3, name=f"sc_{c}_{ibx}_{jt}")
                            jsl = slice(jt * P, (jt + 1) * P)
                            nc.tensor.matmul(
                                sct[:, 0:SC],
                                lhsT=kt[c][0:DK, jsl],
                                rhs=qt[c][0:DK, isl],
                                start=True, stop=True)
                            nc.tensor.matmul(
                                sct[:, SC:2 * SC],
                                lhsT=kt[c][DK:P, jsl],
                                rhs=qt[c][DK:P, isl],
                                start=True, stop=True)
                            et = stream.tile([P, 2 * SC], cdt, tag="e",
                                             bufs=5, name=f"e_{c}_{ibx}_{jt}")
                            if jt % 2 == 0 or DBG_EXP_ACT:
                                nc.scalar.activation(et[:], sct[:], Exp,
                                                     bias=0.0, scale=scale)
                            else:
                                nc.vector.tensor_scalar(
                                    et[:].bitcast(i16), sct[:],
                                    SCH_A, SCH_B, Mult, Add)
                            e_ts[jt] = et
                        for sub in range(2):
                            pj = 2 * (jg - 1) + sub
                            if pj < 0:
                                continue
                            et = e_ts.pop(pj)
                            st, sp = (pj == 0), (pj == JT - 1)
                            for h in range(HPC):
                                nc.tensor.matmul(
                                    av[0:DK + 1, h * SC:(h + 1) * SC],
                                    lhsT=v_c[c][:, pj, h, :],
                                    rhs=et[:, h * SC:(h + 1) * SC],
                                    start=st, stop=sp)
                    # ---- evacuate av (frees the psum bank; av bufs=1),
                    # split across ACT+DVE so it completes quickly ----
                    av_sb = stream.tile([P, 2 * SC], f32, tag="avsb", bufs=2,
                                        name=f"avsb_{c}_{ibx}")
                    nc.scalar.copy(av_sb[0:DK + 1, 0:SC], av[0:DK + 1, 0:SC])
                    nc.vector.tensor_copy(av_sb[0:DK + 1, SC:2 * SC],
                                          av[0:DK + 1, SC:2 * SC])

                    def make_norm(c=c, ibx=ibx, av_sb=av_sb, isl=isl):
                        def norm():
                            # Z rows DMA through DRAM reshaped [128, 8] for
                            # a cheap all-lane reciprocal; mults on GPSIMD.
                            z_d = dscratch.tile([1, 2 * SC], f32, tag="zd",
                                                bufs=2, name=f"zd_{c}_{ibx}")
                            nc.sync.dma_start(z_d[:], av_sb[DK:DK + 1, :])
                            zc = stream.tile([P, 2 * (2 * SC) // P], f32,
                                             tag="zc", bufs=2,
                                             name=f"zc_{c}_{ibx}")
                            zw = (2 * SC) // P
                            nc.sync.dma_start(
                                zc[:, 0:zw],
                                z_d[:, :].rearrange("o (p x) -> (o p) x",
                                                    p=P))
                            nc.vector.reciprocal(zc[:, zw:2 * zw],
                                                 zc[:, 0:zw])
                            rz_d = dscratch.tile([1, 2 * SC], f32, tag="rzd",
                                                 bufs=2,
                                                 name=f"rzd_{c}_{ibx}")
                            nc.sync.dma_start(
                                rz_d[:, :].rearrange("o (p x) -> (o p) x",
                                                     p=P),
                                zc[:, zw:2 * zw])
                            rzb = stream.tile([DK, 2 * SC], f32, tag="rzb",
                                              bufs=2, name=f"rzb_{c}_{ibx}")
                            nc.sync.dma_start(
                                rzb[0:DK, :],
                                rz_d[:, :].to_broadcast((DK, 2 * SC)))
                            nc.gpsimd.tensor_mul(on_c[c][0:DK, isl],
                                                 av_sb[0:DK, 0:SC],
                                                 rzb[0:DK, 0:SC])
                            stg = stream.tile([DK, SC], cdt, tag="stgB",
                                              bufs=2, name=f"stg_{c}_{ibx}")
                            nc.gpsimd.tensor_mul(stg[0:DK, :],
                                                 av_sb[0:DK, SC:2 * SC],
                                                 rzb[0:DK, SC:2 * SC])
                            nc.sync.dma_start(on_c[c][DK:P, isl],
                                              stg[0:DK, :])
                        return norm

                    make_norm()()

            # ---- output projection (bias added on host) ----
            # i-outer so only the last i-chunk waits on the final normalize;
            # per-n staging halves so outT stores are 256KB each.
            o_stgs = [stream.tile([P, 2 * SC], cdt, tag="ostg", bufs=NOUT,
                                  name=f"ostg_{n}") for n in range(NOUT)]
            for i in range(NSC):
                for n in range(NOUT):
                    idx = i * NOUT + n
                    pso = psum.tile([P, 2 * SC], f32, tag="sc", bufs=3,
                                    name=f"ps_o_{n}_{i}")
                    for c in range(HC):
                        nc.tensor.matmul(
                            pso[:, 0:SC],
                            lhsT=wo_sb[:, c, n * P:(n + 1) * P],
                            rhs=on_c[c][:, i * SC:(i + 1) * SC],
                            start=(c == 0), stop=(c == HC - 1))
                    osl = slice((i % 2) * SC, (i % 2 + 1) * SC)
                    if idx % 2 == 0:
                        nc.scalar.copy(o_stgs[n][:, osl], pso[:, 0:SC])
                    else:
                        nc.vector.tensor_copy(o_stgs[n][:, osl], pso[:, 0:SC])
                    if i % 2 == 1:
                        nc.sync.dma_start(
                            outT[n * P:(n + 1) * P,
                                 (i - 1) * SC:(i + 1) * SC],
                            o_stgs[n][:])

    nc.finalize()
    return nc


def make_in_maps(query, key, value, Wq, bq, Wk, bk, Wv, bv, Wo, bo):
    """Shard full inputs into the 8 per-core input dicts."""
    import ml_dtypes
    f = lambda a: np.ascontiguousarray(np.asarray(a, dtype=np.float32))
    HC = DH // P
    query, key, value = f(query), f(key), f(value)
    Wq, Wk, Wv, Wo = f(Wq), f(Wk), f(Wv), f(Wo)
    bq, bk, bv = f(bq), f(bk), f(bv)
    cvt = lambda a: np.ascontiguousarray(a.astype(ml_dtypes.bfloat16))
    in_maps = []
    for core in range(N_CORES):
        b, g = core // GROUPS, core % GROUPS
        sl = slice(g * DH, (g + 1) * DH)
        in_maps.append({
            "qT": cvt(query[b].T),
            "kTd": cvt(key[b].T),
            "vT": cvt(value[b].T),
            "wq": cvt(Wq[:, sl]),
            "wk": cvt(Wk[:, sl]),
            "wv": cvt(Wv[:, sl]),
            "wo": cvt(Wo[sl, :]),
            "bq": np.ascontiguousarray(bq[sl].reshape(HC, P).T),
            "bk": np.ascontiguousarray(bk[sl].reshape(HC, P).T),
            "bvb": np.ascontiguousarray(
                np.broadcast_to(bv[sl].reshape(H_CORE, DK)[None],
                                (P, H_CORE, DK))),
        })
    return in_maps


# test hooks (ignored by the harness)
TRACE = False
LAST_RESULT = None
DTYPE = "bf16"
_NC_CACHE = {}


def kernel(query, key, value, Wq, bq, Wk, bk, Wv, bv, Wo, bo):
    global LAST_RESULT
    from concourse.bass_utils import run_bass_kernel_spmd

    if "nc" not in _NC_CACHE:
        _NC_CACHE["nc"] = build_nc()
    nc = _NC_CACHE["nc"]

    in_maps = make_in_maps(query, key, value, Wq, bq, Wk, bk, Wv, bv, Wo, bo)
    kwargs = {}
    if TRACE:
        kwargs = dict(trace=True, trace_cores=[0])
    res = run_bass_kernel_spmd(nc, in_maps, core_ids=list(range(N_CORES)),
                               **kwargs)
    LAST_RESULT = res

    out = np.zeros((B, S, D), np.float32)
    for core in range(N_CORES):
        b = core // GROUPS
        out[b] += res.results[core]["outT"].T.astype(np.float32)
    out += np.asarray(bo, dtype=np.float32)
    return out
